# revision 24
# baseline (speedup 1.0000x reference)
"""CoAttentionNetwork Trainium2 kernel — 8-core data parallel over batch.

Takes FULL inputs (B=64), shards batch across 8 NeuronCores (8 batches per
core), runs a Bass/Tile kernel per core, gathers per-core outputs.

Per-batch device algorithm (b = one of 8 local batches):
  CWlT[D',t] = Wl^T C^T            (bf16 matmuls, fp32 psum)
  F[t,n]    = tanh(CWlT^T S^T)     -> fp8 e4m3, DoubleRow layout [128,2,2,N]
  FT[n,t]   = PE transpose of fp8 F -> fp8 DR layout [128,4,2,T]
  WcC row [2,T] bf16 mms; snapshot fp8 -> transpose -> wcct DR weights
  Hs[2,n]   = tanh(Ws S^T + (Wc C^T) F)    G via fp8 DoubleRow (2 mm/half)
  Hc[2,t]   = tanh(Wc C^T + (Ws S^T) F^T)  T2 via fp8 DoubleRow (4 mm)
  logits -> exp col layout [128, chunks] with fused accum_out row-sums
  co_s = S^T exp_s, co_c = C^T exp_c; scaled by 1/sum on copy
  out[b] = fc_w @ [co_s; co_c] + fc_b

Emission interleaves batch b's heavy stages (A1..A4) with batch b-1's
dependent tail stages (B1..B4) so the PE queue alternates big matmuls with
small feeder matmuls, giving vector/scalar/gpsimd copies time to land.
"""

import numpy as np

B, N, T, D, K, OUT = 64, 1024, 512, 384, 2, 6
N_CORES = 8
BPC = B // N_CORES  # batches per core
P = 128
NCH = N // P   # 8 n-chunks
TCH = T // P   # 4 t-chunks
DCH = D // P   # 3 d-chunks
FCH = 2 * D // P  # 6 chunks of concat dim
NBLK = NCH // 2  # 4 DoubleRow blocks over n
TBLK = TCH // 2  # 2 DoubleRow blocks over t

_BUILT = {}


def _build_nc():
    import concourse.bacc as bacc
    import concourse.mybir as mybir
    import concourse.tile as tile

    f32 = mybir.dt.float32
    bf16 = mybir.dt.bfloat16
    fp8 = mybir.dt.float8e4
    AF = mybir.ActivationFunctionType
    PM = mybir.MatmulPerfMode

    nc = bacc.Bacc(None, target_bir_lowering=False, debug=False)

    S_d = nc.dram_tensor("S", [BPC, N, D], bf16, kind="ExternalInput")
    ST_d = nc.dram_tensor("ST", [BPC, D, N], bf16, kind="ExternalInput")
    C_d = nc.dram_tensor("C", [BPC, T, D], bf16, kind="ExternalInput")
    CT_d = nc.dram_tensor("CT", [BPC, D, T], bf16, kind="ExternalInput")
    Wl_d = nc.dram_tensor("Wl", [D, D], bf16, kind="ExternalInput")
    WsT_d = nc.dram_tensor("WsT", [D, K], bf16, kind="ExternalInput")
    WcT_d = nc.dram_tensor("WcT", [D, K], bf16, kind="ExternalInput")
    whsbd_d = nc.dram_tensor("whsbd", [32 + K, K], bf16, kind="ExternalInput")
    whcbd_d = nc.dram_tensor("whcbd", [32 + K, K], bf16, kind="ExternalInput")
    eye2_d = nc.dram_tensor("eye2", [K, K], fp8, kind="ExternalInput")
    ident_d = nc.dram_tensor("ident", [P, P], fp8, kind="ExternalInput")
    fcwT_d = nc.dram_tensor("fcwT", [2 * D, OUT], bf16, kind="ExternalInput")
    fcb_d = nc.dram_tensor("fcb", [1, OUT], f32, kind="ExternalInput")
    ones_d = nc.dram_tensor("ones", [P, 1], f32, kind="ExternalInput")
    onesb_d = nc.dram_tensor("onesb", [1, P], bf16, kind="ExternalInput")
    out_d = nc.dram_tensor("out", [1, BPC * OUT], f32, kind="ExternalOutput")

    with tile.TileContext(nc) as tc:
        with (
            tc.tile_pool(name="wpool", bufs=1) as wpool,
            tc.tile_pool(name="io", bufs=3) as io,
            tc.tile_pool(name="work", bufs=2) as work,
            tc.tile_pool(name="pbig", bufs=3, space="PSUM") as pbig,
            tc.tile_pool(name="pft", bufs=2, space="PSUM") as pft,
            tc.tile_pool(name="prow", bufs=3, space="PSUM") as prow,
        ):
            # ---- constants / weights (loaded once) ----
            wl_sb = wpool.tile([P, DCH, D], bf16)
            nc.gpsimd.dma_start(wl_sb[:], Wl_d.rearrange("(c p) m -> p c m", p=P))
            wst_sb = wpool.tile([P, DCH, K], bf16)
            nc.gpsimd.dma_start(wst_sb[:], WsT_d.rearrange("(c p) k -> p c k", p=P))
            wct_sb = wpool.tile([P, DCH, K], bf16)
            nc.gpsimd.dma_start(wct_sb[:], WcT_d.rearrange("(c p) k -> p c k", p=P))
            whsbd_sb = wpool.tile([32 + K, K], bf16)
            nc.gpsimd.dma_start(whsbd_sb[:], whsbd_d[:])
            whcbd_sb = wpool.tile([32 + K, K], bf16)
            nc.gpsimd.dma_start(whcbd_sb[:], whcbd_d[:])
            # persistent paired-row tiles: even batch at partitions 0:2,
            # odd batch at 32:34; zeroed once so garbage partitions read 0
            hs2_ab = [wpool.tile([32 + K, N], bf16, name=f"hs2_{i}")
                      for i in range(2)]
            hc2_ab = [wpool.tile([32 + K, T], bf16, name=f"hc2_{i}")
                      for i in range(2)]
            for t_ in hs2_ab + hc2_ab:
                nc.vector.memset(t_[:], 0.0)
            eye2_sb = wpool.tile([K, K], fp8)
            nc.gpsimd.dma_start(eye2_sb[:], eye2_d[:])
            ident_sb = wpool.tile([P, P], fp8)
            nc.gpsimd.dma_start(ident_sb[:], ident_d[:])
            fcw_sb = wpool.tile([P, FCH, OUT], bf16)
            nc.gpsimd.dma_start(fcw_sb[:], fcwT_d.rearrange("(c p) o -> p c o", p=P))
            fcb_sb = wpool.tile([1, OUT], f32)
            nc.gpsimd.dma_start(fcb_sb[:], fcb_d[:])
            ones_sb = wpool.tile([P, 1], f32)
            nc.gpsimd.dma_start(ones_sb[:], ones_d[:])
            onesb_sb = wpool.tile([1, P], bf16)
            nc.gpsimd.dma_start(onesb_sb[:], onesb_d[:])
            out_sb = wpool.tile([1, BPC * OUT], f32)

            def stageA1(b):
                # input DMAs + CWlT
                ct = io.tile([P, DCH, T], bf16)
                nc.sync.dma_start(ct[:], CT_d[b].rearrange("(c p) t -> p c t", p=P))
                st = io.tile([P, DCH, N], bf16)
                nc.sync.dma_start(st[:], ST_d[b].rearrange("(c p) n -> p c n", p=P))
                s_nat = io.tile([P, NCH, D], bf16)
                nc.gpsimd.dma_start(s_nat[:], S_d[b].rearrange("(c p) d -> p c d", p=P))
                c_nat = io.tile([P, TCH, D], bf16)
                nc.gpsimd.dma_start(c_nat[:], C_d[b].rearrange("(c p) d -> p c d", p=P))

                cwlt = work.tile([P, DCH, T], bf16)
                for dc in range(DCH):
                    pb = pbig.tile([P, 512], f32, tag="pbig")
                    for kd in range(DCH):
                        nc.tensor.matmul(
                            pb[:],
                            wl_sb[:, kd, dc * P:(dc + 1) * P],
                            ct[:, kd, :],
                            start=(kd == 0), stop=(kd == DCH - 1),
                        )
                    nc.vector.tensor_copy(cwlt[:, dc, :], pb[:])
                f_dr = work.tile([P, TBLK, 2, N], fp8)
                ft_dr = work.tile([P, NBLK, 2, T], fp8)
                return dict(s_nat=s_nat, st=st, c_nat=c_nat, ct=ct,
                            cwlt=cwlt, f_dr=f_dr, ft_dr=ft_dr)

            def stageAF(b, tl, tcs):
                # F [t, n] = tanh(CWlT^T @ ST) -> fp8 DR layout, t-chunks tcs
                cwlt, st, f_dr = tl["cwlt"], tl["st"], tl["f_dr"]
                for tcI in tcs:
                    pb0 = pbig.tile([P, 512], f32, tag="pbig")
                    pb1 = pbig.tile([P, 512], f32, tag="pbig")
                    for kd in range(DCH):
                        lhs = cwlt[:, kd, tcI * P:(tcI + 1) * P]
                        nc.tensor.matmul(
                            pb0[:], lhs, st[:, kd, 0:512],
                            start=(kd == 0), stop=(kd == DCH - 1))
                        nc.tensor.matmul(
                            pb1[:], lhs, st[:, kd, 512:1024],
                            start=(kd == 0), stop=(kd == DCH - 1))
                    blk, kt = tcI // 2, tcI % 2
                    nc.scalar.activation(f_dr[:, blk, kt, 0:512], pb0[:], AF.Tanh)
                    nc.scalar.activation(f_dr[:, blk, kt, 512:1024], pb1[:], AF.Tanh)

            def stageAT(b, tl):
                # FT via PE transpose of fp8 F; one psum slot per n-chunk PAIR
                f_dr, ft_dr = tl["f_dr"], tl["ft_dr"]
                for pr in range(NBLK):
                    pb = pft.tile([P, 2, 512, 2], fp8, tag="pft")
                    for h in range(2):
                        ncI = 2 * pr + h
                        for tcI in range(TCH):
                            nc.tensor.transpose(
                                pb[:, h, tcI * P:(tcI + 1) * P, 0:1],
                                f_dr[:, tcI // 2, tcI % 2,
                                     ncI * P:(ncI + 1) * P],
                                ident_sb[:])
                    nc.vector.tensor_copy(ft_dr[:, pr, :, :], pb[:, :, :, 0])

            def stageB1(b, tl):
                # Hc part 1: WcC row; snapshot fp8; wcct DR weights
                ct = tl["ct"]
                hcp = prow.tile([K, T], f32, tag="prow")
                tl["hcp"] = hcp
                for kd in range(DCH):
                    nc.tensor.matmul(
                        hcp[:], wct_sb[:, kd, :], ct[:, kd, :],
                        start=(kd == 0), stop=False)
                wcc8 = work.tile([K, T], fp8, tag="wcc8")
                nc.vector.tensor_copy(wcc8[:], hcp[:])
                pwt = pft.tile([P, TBLK, 2, K, 2], fp8, tag="pft")
                for tcI in range(TCH):
                    nc.tensor.transpose(
                        pwt[:, tcI // 2, tcI % 2, :, 0:1],
                        wcc8[:, tcI * P:(tcI + 1) * P], eye2_sb[:])
                wcct_dr = work.tile([P, TBLK, 2, 16], fp8, tag="wcct_dr")
                nc.vector.tensor_copy(wcct_dr[:, :, :, 0:K], pwt[:, :, :, :, 0])
                tl["wcct_dr"] = wcct_dr

            def stageB2(b, tl, pair):
                # Hs rows = tanh(WsS + G); wsst DR weights from snapshots
                st, f_dr = tl["st"], tl["f_dr"]
                wcct_dr = tl["wcct_dr"]
                wss8 = work.tile([K, N], fp8, tag="wss8")
                if b % 2 == 0:
                    pair["hs2"] = hs2_ab[(b // 2) % 2]
                    pair["hc2"] = hc2_ab[(b // 2) % 2]
                hs2 = pair["hs2"]
                ro = 32 * (b % 2)
                for nh in range(2):
                    sl = slice(nh * 512, (nh + 1) * 512)
                    ph = prow.tile([K, 512], f32, tag="prow")
                    for kd in range(DCH):
                        nc.tensor.matmul(
                            ph[:], wst_sb[:, kd, :], st[:, kd, sl],
                            start=(kd == 0), stop=False)
                    if nh == 0:
                        nc.scalar.activation(wss8[:, sl], ph[:], AF.Copy)
                    else:
                        nc.vector.tensor_copy(wss8[:, sl], ph[:])
                    for blk in range(TBLK):
                        nc.tensor.matmul(
                            ph[:], wcct_dr[:, blk, :, 0:K],
                            f_dr[:, blk, :, sl],
                            start=False, stop=(blk == TBLK - 1),
                            perf_mode=PM.DoubleRow)
                    nc.scalar.activation(hs2[ro:ro + K, sl], ph[:], AF.Tanh)

                pnt = pft.tile([P, NBLK, 2, K, 2], fp8, tag="pft")
                for ncI in range(NCH):
                    nc.tensor.transpose(
                        pnt[:, ncI // 2, ncI % 2, :, 0:1],
                        wss8[:, ncI * P:(ncI + 1) * P], eye2_sb[:])
                wsst_dr = work.tile([P, NBLK, 2, 16], fp8, tag="wsst_dr")
                nc.vector.tensor_copy(wsst_dr[:, :, :, 0:K], pnt[:, :, :, :, 0])
                tl["wsst_dr"] = wsst_dr

            def stageB3(b, tl, pair):
                # Hc part 2 (T2 DoubleRow) -> paired hc2 rows
                hcp, ft_dr, wsst_dr = tl["hcp"], tl["ft_dr"], tl["wsst_dr"]
                for blk in range(NBLK):
                    nc.tensor.matmul(
                        hcp[:], wsst_dr[:, blk, :, 0:K],
                        ft_dr[:, blk, :, :],
                        start=False, stop=(blk == NBLK - 1),
                        perf_mode=PM.DoubleRow)
                ro = 32 * (b % 2)
                nc.scalar.activation(pair["hc2"][ro:ro + K, :], hcp[:], AF.Tanh)

            def stageBlog(pair, tle, tlo):
                # paired logits via block-diag weights, exp with fused sums
                hs2, hc2 = pair["hs2"], pair["hc2"]
                plog = prow.tile([P, NCH + TCH, K], f32, tag="prow")
                for ncI in range(NCH):
                    nc.tensor.matmul(
                        plog[:, ncI, :],
                        hs2[:, ncI * P:(ncI + 1) * P], whsbd_sb[:],
                        start=True, stop=True)
                for tcI in range(TCH):
                    nc.tensor.matmul(
                        plog[:, NCH + tcI, :],
                        hc2[:, tcI * P:(tcI + 1) * P], whcbd_sb[:],
                        start=True, stop=True)
                for be, tl in ((0, tle), (1, tlo)):
                    es = work.tile([P, NCH], bf16, tag="es")
                    ec = work.tile([P, TCH], bf16, tag="ec")
                    rsrc = work.tile([P, 2], f32, tag="rsrc")
                    nc.scalar.activation(es[:], plog[:, 0:NCH, be], AF.Exp,
                                         accum_out=rsrc[:, 0:1])
                    nc.scalar.activation(ec[:], plog[:, NCH:NCH + TCH, be],
                                         AF.Exp, accum_out=rsrc[:, 1:2])
                    tl["es"], tl["ec"], tl["rsrc"] = es, ec, rsrc

            def stageB4(b, tl):
                # sums -> reciprocal, co vectors, fc output
                s_nat, c_nat = tl["s_nat"], tl["c_nat"]
                es, ec, rsrc = tl["es"], tl["ec"], tl["rsrc"]
                ps2 = prow.tile([1, 2], f32, tag="prow")
                nc.tensor.matmul(ps2[:, 0:1], rsrc[:, 0:1], ones_sb[:],
                                 start=True, stop=True)
                nc.tensor.matmul(ps2[:, 1:2], rsrc[:, 1:2], ones_sb[:],
                                 start=True, stop=True)
                rinv = work.tile([1, 2], f32, tag="rinv")
                nc.vector.reciprocal(rinv[:], ps2[:])

                pco_s = prow.tile([1, D], f32, tag="prow")
                for ncI in range(NCH):
                    nc.tensor.matmul(
                        pco_s[:], es[:, ncI:ncI + 1], s_nat[:, ncI, :],
                        start=(ncI == 0), stop=(ncI == NCH - 1))
                pco_c = prow.tile([1, D], f32, tag="prow")
                for tcI in range(TCH):
                    nc.tensor.matmul(
                        pco_c[:], ec[:, tcI:tcI + 1], c_nat[:, tcI, :],
                        start=(tcI == 0), stop=(tcI == TCH - 1))

                co_row = work.tile([1, 2 * D], bf16, tag="co_row")
                nc.scalar.activation(co_row[:, 0:D], pco_s[:], AF.Copy,
                                     scale=rinv[:, 0:1])
                nc.vector.tensor_scalar_mul(co_row[:, D:2 * D], pco_c[:],
                                            rinv[:, 1:2])

                pcol = prow.tile([P, FCH], f32, tag="prow")
                for j in range(FCH):
                    nc.tensor.matmul(
                        pcol[:, j:j + 1], co_row[:, j * P:(j + 1) * P],
                        onesb_sb[0:1, 0:1], start=True, stop=True)
                ccol = work.tile([P, FCH], bf16, tag="ccol")
                nc.vector.tensor_copy(ccol[:], pcol[:])

                pout = prow.tile([1, OUT], f32, tag="prow")
                for j in range(FCH):
                    nc.tensor.matmul(
                        pout[:], ccol[:, j:j + 1], fcw_sb[:, j, :],
                        start=(j == 0), stop=(j == FCH - 1))
                nc.vector.tensor_add(out_sb[:, b * OUT:(b + 1) * OUT],
                                     pout[:], fcb_sb[:])

            # ---- software pipeline: batch b heavy stages interleaved with
            # batch b-1 tail stages; logits/exp/co run per batch-PAIR ----
            prev = None
            prev2 = None
            pair = {}
            for b in range(BPC):
                tl = stageA1(b)
                if prev is not None:
                    stageB1(b - 1, prev)
                stageAF(b, tl, (0, 1))
                if prev is not None:
                    stageB2(b - 1, prev, pair)
                stageAF(b, tl, (2, 3))
                if prev is not None:
                    stageB3(b - 1, prev, pair)
                stageAT(b, tl)
                if prev is not None and (b - 1) % 2 == 1:
                    stageBlog(pair, prev2, prev)
                    stageB4(b - 2, prev2)
                    stageB4(b - 1, prev)
                prev2 = prev
                prev = tl
            b = BPC - 1
            stageB1(b, prev)
            stageB2(b, prev, pair)
            stageB3(b, prev, pair)
            stageBlog(pair, prev2, prev)
            stageB4(b - 1, prev2)
            stageB4(b, prev)
            nc.sync.dma_start(out_d[:], out_sb[:])

    nc.compile()
    return nc


def _get_nc():
    if "nc" not in _BUILT:
        _BUILT["nc"] = _build_nc()
    return _BUILT["nc"]


def _blockdiag2(wh, dtype):
    # [1,K] row weights -> [34, K] block-diagonal for paired-batch logits
    # (even batch rows 0:2 -> col 0, odd batch rows 32:34 -> col 1)
    bd = np.zeros((32 + K, K), dtype=np.float32)
    bd[0:K, 0] = wh[0]
    bd[32:32 + K, 1] = wh[0]
    return np.ascontiguousarray(bd.astype(dtype))


def kernel(S, C, Wl, Ws, Wc, Whs, Whc, fc_w, fc_b):
    import ml_dtypes
    from concourse.bass_utils import run_bass_kernel_spmd

    bf = ml_dtypes.bfloat16
    f8 = ml_dtypes.float8_e4m3fn
    S = np.ascontiguousarray(np.asarray(S, dtype=np.float32))
    C = np.ascontiguousarray(np.asarray(C, dtype=np.float32))
    Wl = np.asarray(Wl, dtype=np.float32)
    Ws = np.asarray(Ws, dtype=np.float32)
    Wc = np.asarray(Wc, dtype=np.float32)
    Whs = np.asarray(Whs, dtype=np.float32)
    Whc = np.asarray(Whc, dtype=np.float32)
    fc_w = np.asarray(fc_w, dtype=np.float32)
    fc_b = np.asarray(fc_b, dtype=np.float32)

    nc = _get_nc()

    in_common = {
        "Wl": np.ascontiguousarray(Wl.astype(bf)),
        "WsT": np.ascontiguousarray(Ws.T.astype(bf)),
        "WcT": np.ascontiguousarray(Wc.T.astype(bf)),
        "whsbd": _blockdiag2(Whs, bf),
        "whcbd": _blockdiag2(Whc, bf),
        "eye2": np.eye(K, dtype=f8),
        "ident": np.eye(P, dtype=f8),
        "fcwT": np.ascontiguousarray(fc_w.T.astype(bf)),
        "fcb": np.ascontiguousarray(fc_b[None, :]),
        "ones": np.ones((P, 1), dtype=np.float32),
        "onesb": np.ones((1, P), dtype=bf),
    }
    in_maps = []
    for i in range(N_CORES):
        sl = slice(i * BPC, (i + 1) * BPC)
        in_maps.append({
            "S": np.ascontiguousarray(S[sl].astype(bf)),
            "ST": np.ascontiguousarray(S[sl].transpose(0, 2, 1).astype(bf)),
            "C": np.ascontiguousarray(C[sl].astype(bf)),
            "CT": np.ascontiguousarray(C[sl].transpose(0, 2, 1).astype(bf)),
            **in_common,
        })

    _BUILT["last_in_maps"] = in_maps
    res = run_bass_kernel_spmd(nc, in_maps, list(range(N_CORES)))
    return np.concatenate(
        [res.results[i]["out"].reshape(BPC, OUT) for i in range(N_CORES)], axis=0)


def __getattr__(name):
    if name == "_LAST_IN_MAPS":
        return _BUILT["last_in_maps"]
    raise AttributeError(name)


# revision 33
# speedup vs baseline: 1.0671x; 1.0671x over previous
"""CoAttentionNetwork Trainium2 kernel — 8-core data parallel over batch.

Takes FULL inputs (B=64), shards batch across 8 NeuronCores (8 batches per
core), runs a Bass/Tile kernel per core, gathers per-core outputs.

Per-batch device algorithm (b = one of 8 local batches):
  CWlT[D',t] = Wl^T C^T            (bf16 matmuls, fp32 psum)
  F[t,n]    = tanh(CWlT^T S^T)     -> fp8 e4m3, DoubleRow layout [128,2,2,N]
  FT[n,t]   = PE transpose of fp8 F -> fp8 DR layout [128,4,2,T]
  WcC row [2,T] bf16 mms; snapshot fp8 -> transpose -> wcct DR weights
  Hs[2,n]   = tanh(Ws S^T + (Wc C^T) F)    G via fp8 DoubleRow (2 mm/half)
  Hc[2,t]   = tanh(Wc C^T + (Ws S^T) F^T)  T2 via fp8 DoubleRow (4 mm)
  logits -> exp col layout [128, chunks] with fused accum_out row-sums
  co_s = S^T exp_s, co_c = C^T exp_c; scaled by 1/sum on copy
  out[b] = fc_w @ [co_s; co_c] + fc_b

Emission interleaves batch b's heavy stages (A1..A4) with batch b-1's
dependent tail stages (B1..B4) so the PE queue alternates big matmuls with
small feeder matmuls, giving vector/scalar/gpsimd copies time to land.
"""

import numpy as np

B, N, T, D, K, OUT = 64, 1024, 512, 384, 2, 6
N_CORES = 8
BPC = B // N_CORES  # batches per core
P = 128
NCH = N // P   # 8 n-chunks
TCH = T // P   # 4 t-chunks
DCH = D // P   # 3 d-chunks
FCH = 2 * D // P  # 6 chunks of concat dim
NBLK = NCH // 2  # 4 DoubleRow blocks over n
TBLK = TCH // 2  # 2 DoubleRow blocks over t

_BUILT = {}


def _build_nc():
    import concourse.bacc as bacc
    import concourse.mybir as mybir
    import concourse.tile as tile

    f32 = mybir.dt.float32
    bf16 = mybir.dt.bfloat16
    fp8 = mybir.dt.float8e4
    AF = mybir.ActivationFunctionType
    PM = mybir.MatmulPerfMode

    nc = bacc.Bacc(None, target_bir_lowering=False, debug=False)

    S_d = nc.dram_tensor("S", [BPC, N, D], bf16, kind="ExternalInput")
    ST_d = nc.dram_tensor("ST", [BPC, D, N], bf16, kind="ExternalInput")
    C_d = nc.dram_tensor("C", [BPC, T, D], bf16, kind="ExternalInput")
    CT_d = nc.dram_tensor("CT", [BPC, D, T], bf16, kind="ExternalInput")
    Wl_d = nc.dram_tensor("Wl", [D, D], bf16, kind="ExternalInput")
    WsT_d = nc.dram_tensor("WsT", [D, K], bf16, kind="ExternalInput")
    WcT_d = nc.dram_tensor("WcT", [D, K], bf16, kind="ExternalInput")
    whsT_d = nc.dram_tensor("whsT", [K, 1], bf16, kind="ExternalInput")
    whcT_d = nc.dram_tensor("whcT", [K, 1], bf16, kind="ExternalInput")
    eye2_d = nc.dram_tensor("eye2", [K, K], fp8, kind="ExternalInput")
    ident_d = nc.dram_tensor("ident", [P, P], fp8, kind="ExternalInput")
    fcwT_d = nc.dram_tensor("fcwT", [2 * D, OUT], bf16, kind="ExternalInput")
    fcb_d = nc.dram_tensor("fcb", [1, OUT], f32, kind="ExternalInput")
    ones_d = nc.dram_tensor("ones", [P, 1], f32, kind="ExternalInput")
    onesb_d = nc.dram_tensor("onesb", [1, P], bf16, kind="ExternalInput")
    out_d = nc.dram_tensor("out", [1, BPC * OUT], f32, kind="ExternalOutput")

    with tile.TileContext(nc) as tc:
        with (
            tc.tile_pool(name="wpool", bufs=1) as wpool,
            tc.tile_pool(name="io", bufs=3) as io,
            tc.tile_pool(name="work", bufs=2) as work,
            tc.tile_pool(name="pbig", bufs=3, space="PSUM") as pbig,
            tc.tile_pool(name="pft", bufs=2, space="PSUM") as pft,
            tc.tile_pool(name="prow", bufs=3, space="PSUM") as prow,
        ):
            # ---- constants / weights (loaded once) ----
            wl_sb = wpool.tile([P, DCH, D], bf16)
            nc.gpsimd.dma_start(wl_sb[:], Wl_d.rearrange("(c p) m -> p c m", p=P))
            wst_sb = wpool.tile([P, DCH, K], bf16)
            nc.gpsimd.dma_start(wst_sb[:], WsT_d.rearrange("(c p) k -> p c k", p=P))
            wct_sb = wpool.tile([P, DCH, K], bf16)
            nc.gpsimd.dma_start(wct_sb[:], WcT_d.rearrange("(c p) k -> p c k", p=P))
            whst_sb = wpool.tile([K, 1], bf16)
            nc.gpsimd.dma_start(whst_sb[:], whsT_d[:])
            whct_sb = wpool.tile([K, 1], bf16)
            nc.gpsimd.dma_start(whct_sb[:], whcT_d[:])
            eye2_sb = wpool.tile([K, K], fp8)
            nc.gpsimd.dma_start(eye2_sb[:], eye2_d[:])
            ident_sb = wpool.tile([P, P], fp8)
            nc.gpsimd.dma_start(ident_sb[:], ident_d[:])
            fcw_sb = wpool.tile([P, FCH, OUT], bf16)
            nc.gpsimd.dma_start(fcw_sb[:], fcwT_d.rearrange("(c p) o -> p c o", p=P))
            fcb_sb = wpool.tile([1, OUT], f32)
            nc.gpsimd.dma_start(fcb_sb[:], fcb_d[:])
            ones_sb = wpool.tile([P, 1], f32)
            nc.gpsimd.dma_start(ones_sb[:], ones_d[:])
            onesb_sb = wpool.tile([1, P], bf16)
            nc.gpsimd.dma_start(onesb_sb[:], onesb_d[:])
            out_sb = wpool.tile([1, BPC * OUT], f32)

            def stageA1(b):
                # input DMAs + CWlT
                ct = io.tile([P, DCH, T], bf16)
                nc.sync.dma_start(ct[:], CT_d[b].rearrange("(c p) t -> p c t", p=P))
                st = io.tile([P, DCH, N], bf16)
                nc.sync.dma_start(st[:], ST_d[b].rearrange("(c p) n -> p c n", p=P))
                s_nat = io.tile([P, NCH, D], bf16)
                nc.sync.dma_start(s_nat[:], S_d[b].rearrange("(c p) d -> p c d", p=P))
                c_nat = io.tile([P, TCH, D], bf16)
                nc.sync.dma_start(c_nat[:], C_d[b].rearrange("(c p) d -> p c d", p=P))

                cwlt = work.tile([P, DCH, T], bf16)
                for dc in range(DCH):
                    pb = pbig.tile([P, 512], f32, tag="pbig")
                    for kd in range(DCH):
                        nc.tensor.matmul(
                            pb[:],
                            wl_sb[:, kd, dc * P:(dc + 1) * P],
                            ct[:, kd, :],
                            start=(kd == 0), stop=(kd == DCH - 1),
                        )
                    nc.vector.tensor_copy(cwlt[:, dc, :], pb[:])
                f_dr = work.tile([P, TBLK, 2, N], fp8)
                ft_dr = work.tile([P, NBLK, 2, T], fp8)
                return dict(s_nat=s_nat, st=st, c_nat=c_nat, ct=ct,
                            cwlt=cwlt, f_dr=f_dr, ft_dr=ft_dr)

            def stageAF(b, tl, tcs):
                # F [t, n] = tanh(CWlT^T @ ST) -> fp8 DR layout, t-chunks tcs
                cwlt, st, f_dr = tl["cwlt"], tl["st"], tl["f_dr"]
                for tcI in tcs:
                    pb0 = pbig.tile([P, 512], f32, tag="pbig")
                    pb1 = pbig.tile([P, 512], f32, tag="pbig")
                    for kd in range(DCH):
                        lhs = cwlt[:, kd, tcI * P:(tcI + 1) * P]
                        nc.tensor.matmul(
                            pb0[:], lhs, st[:, kd, 0:512],
                            start=(kd == 0), stop=(kd == DCH - 1))
                        nc.tensor.matmul(
                            pb1[:], lhs, st[:, kd, 512:1024],
                            start=(kd == 0), stop=(kd == DCH - 1))
                    blk, kt = tcI // 2, tcI % 2
                    nc.scalar.activation(f_dr[:, blk, kt, 0:512], pb0[:], AF.Tanh)
                    nc.scalar.activation(f_dr[:, blk, kt, 512:1024], pb1[:], AF.Tanh)

            def stageAT(b, tl):
                # FT via PE transpose of fp8 F; one psum slot per n-chunk PAIR
                f_dr, ft_dr = tl["f_dr"], tl["ft_dr"]
                for pr in range(NBLK):
                    pb = pft.tile([P, 2, 512, 2], fp8, tag="pft")
                    for h in range(2):
                        ncI = 2 * pr + h
                        for tcI in range(TCH):
                            nc.tensor.transpose(
                                pb[:, h, tcI * P:(tcI + 1) * P, 0:1],
                                f_dr[:, tcI // 2, tcI % 2,
                                     ncI * P:(ncI + 1) * P],
                                ident_sb[:])
                    if pr % 2 == 0:
                        nc.scalar.activation(ft_dr[:, pr, :, :], pb[:, :, :, 0],
                                             AF.Copy)
                    else:
                        nc.vector.tensor_copy(ft_dr[:, pr, :, :], pb[:, :, :, 0])

            def stageB1(b, tl):
                # Hc part 1: WcC row; snapshot fp8; wcct DR weights
                ct = tl["ct"]
                hcp = prow.tile([K, T], f32, tag="prow")
                tl["hcp"] = hcp
                for kd in range(DCH):
                    nc.tensor.matmul(
                        hcp[:], wct_sb[:, kd, :], ct[:, kd, :],
                        start=(kd == 0), stop=False)
                wcc8 = work.tile([K, T], fp8, tag="wcc8")
                nc.vector.tensor_copy(wcc8[:], hcp[:])
                pwt = pft.tile([P, TBLK, 2, K, 2], fp8, tag="pft")
                for tcI in range(TCH):
                    nc.tensor.transpose(
                        pwt[:, tcI // 2, tcI % 2, :, 0:1],
                        wcc8[:, tcI * P:(tcI + 1) * P], eye2_sb[:])
                wcct_dr = work.tile([P, TBLK, 2, 16], fp8, tag="wcct_dr")
                nc.vector.tensor_copy(wcct_dr[:, :, :, 0:K], pwt[:, :, :, :, 0])
                tl["wcct_dr"] = wcct_dr

            def stageB2(b, tl):
                # Hs rows = tanh(WsS + G); wsst DR weights from snapshots
                st, f_dr = tl["st"], tl["f_dr"]
                wcct_dr = tl["wcct_dr"]
                wss8 = work.tile([K, N], fp8, tag="wss8")
                hs_row = work.tile([K, N], bf16, tag="hs_row")
                tl["hs_row"] = hs_row
                for nh in range(2):
                    sl = slice(nh * 512, (nh + 1) * 512)
                    ph = prow.tile([K, 512], f32, tag="prow")
                    for kd in range(DCH):
                        nc.tensor.matmul(
                            ph[:], wst_sb[:, kd, :], st[:, kd, sl],
                            start=(kd == 0), stop=False)
                    if nh == 0:
                        nc.scalar.activation(wss8[:, sl], ph[:], AF.Copy)
                    else:
                        nc.vector.tensor_copy(wss8[:, sl], ph[:])
                    for blk in range(TBLK):
                        nc.tensor.matmul(
                            ph[:], wcct_dr[:, blk, :, 0:K],
                            f_dr[:, blk, :, sl],
                            start=False, stop=(blk == TBLK - 1),
                            perf_mode=PM.DoubleRow)
                    nc.scalar.activation(hs_row[:, sl], ph[:], AF.Tanh)

                pnt = pft.tile([P, NBLK, 2, K, 2], fp8, tag="pft")
                for ncI in range(NCH):
                    nc.tensor.transpose(
                        pnt[:, ncI // 2, ncI % 2, :, 0:1],
                        wss8[:, ncI * P:(ncI + 1) * P], eye2_sb[:])
                wsst_dr = work.tile([P, NBLK, 2, 16], fp8, tag="wsst_dr")
                nc.vector.tensor_copy(wsst_dr[:, :, :, 0:K], pnt[:, :, :, :, 0])
                tl["wsst_dr"] = wsst_dr

            def stageB3(b, tl):
                # Hc part 2 (T2 DoubleRow), logits, exp with fused sums
                hcp, ft_dr, wsst_dr = tl["hcp"], tl["ft_dr"], tl["wsst_dr"]
                hs_row = tl["hs_row"]
                for blk in range(NBLK):
                    nc.tensor.matmul(
                        hcp[:], wsst_dr[:, blk, :, 0:K],
                        ft_dr[:, blk, :, :],
                        start=False, stop=(blk == NBLK - 1),
                        perf_mode=PM.DoubleRow)
                hc_row = work.tile([K, T], bf16, tag="hc_row")
                nc.scalar.activation(hc_row[:], hcp[:], AF.Tanh)

                plog = prow.tile([P, NCH + TCH], f32, tag="prow")
                for ncI in range(NCH):
                    nc.tensor.matmul(
                        plog[:, ncI:ncI + 1],
                        hs_row[:, ncI * P:(ncI + 1) * P], whst_sb[:],
                        start=True, stop=True)
                for tcI in range(TCH):
                    nc.tensor.matmul(
                        plog[:, NCH + tcI:NCH + tcI + 1],
                        hc_row[:, tcI * P:(tcI + 1) * P], whct_sb[:],
                        start=True, stop=True)
                es = work.tile([P, NCH], bf16, tag="es")
                ec = work.tile([P, TCH], bf16, tag="ec")
                rsrc = work.tile([P, 2], f32, tag="rsrc")
                nc.scalar.activation(es[:], plog[:, 0:NCH], AF.Exp,
                                     accum_out=rsrc[:, 0:1])
                nc.scalar.activation(ec[:], plog[:, NCH:NCH + TCH], AF.Exp,
                                     accum_out=rsrc[:, 1:2])
                tl["es"], tl["ec"], tl["rsrc"] = es, ec, rsrc

            def stageB4(b, tl):
                # sums -> reciprocal, co vectors, fc output
                s_nat, c_nat = tl["s_nat"], tl["c_nat"]
                es, ec, rsrc = tl["es"], tl["ec"], tl["rsrc"]
                ps2 = prow.tile([1, 2], f32, tag="prow")
                nc.tensor.matmul(ps2[:, 0:1], rsrc[:, 0:1], ones_sb[:],
                                 start=True, stop=True)
                nc.tensor.matmul(ps2[:, 1:2], rsrc[:, 1:2], ones_sb[:],
                                 start=True, stop=True)
                rinv = work.tile([1, 2], f32, tag="rinv")
                nc.vector.reciprocal(rinv[:], ps2[:])

                pco_s = prow.tile([1, D], f32, tag="prow")
                for ncI in range(NCH):
                    nc.tensor.matmul(
                        pco_s[:], es[:, ncI:ncI + 1], s_nat[:, ncI, :],
                        start=(ncI == 0), stop=(ncI == NCH - 1))
                pco_c = prow.tile([1, D], f32, tag="prow")
                for tcI in range(TCH):
                    nc.tensor.matmul(
                        pco_c[:], ec[:, tcI:tcI + 1], c_nat[:, tcI, :],
                        start=(tcI == 0), stop=(tcI == TCH - 1))

                co_row = work.tile([1, 2 * D], bf16, tag="co_row")
                nc.scalar.activation(co_row[:, 0:D], pco_s[:], AF.Copy,
                                     scale=rinv[:, 0:1])
                nc.vector.tensor_scalar_mul(co_row[:, D:2 * D], pco_c[:],
                                            rinv[:, 1:2])

                pcol = prow.tile([P, FCH], f32, tag="prow")
                for j in range(FCH):
                    nc.tensor.matmul(
                        pcol[:, j:j + 1], co_row[:, j * P:(j + 1) * P],
                        onesb_sb[0:1, 0:1], start=True, stop=True)
                ccol = work.tile([P, FCH], bf16, tag="ccol")
                nc.vector.tensor_copy(ccol[:], pcol[:])

                pout = prow.tile([1, OUT], f32, tag="prow")
                for j in range(FCH):
                    nc.tensor.matmul(
                        pout[:], ccol[:, j:j + 1], fcw_sb[:, j, :],
                        start=(j == 0), stop=(j == FCH - 1))
                nc.vector.tensor_add(out_sb[:, b * OUT:(b + 1) * OUT],
                                     pout[:], fcb_sb[:])

            # ---- software pipeline: batch b heavy stages interleaved with
            # batch b-1 tail stages ----
            prev = None
            for b in range(BPC):
                tl = stageA1(b)
                if prev is not None:
                    stageB1(b - 1, prev)
                stageAF(b, tl, (0, 1))
                if prev is not None:
                    stageB2(b - 1, prev)
                stageAF(b, tl, (2, 3))
                if prev is not None:
                    stageB3(b - 1, prev)
                stageAT(b, tl)
                if prev is not None:
                    stageB4(b - 1, prev)
                prev = tl
            stageB1(BPC - 1, prev)
            stageB2(BPC - 1, prev)
            stageB3(BPC - 1, prev)
            stageB4(BPC - 1, prev)
            nc.sync.dma_start(out_d[:], out_sb[:])

    nc.compile()
    return nc


def _get_nc():
    if "nc" not in _BUILT:
        _BUILT["nc"] = _build_nc()
    return _BUILT["nc"]


def kernel(S, C, Wl, Ws, Wc, Whs, Whc, fc_w, fc_b):
    import ml_dtypes
    from concourse.bass_utils import run_bass_kernel_spmd

    bf = ml_dtypes.bfloat16
    f8 = ml_dtypes.float8_e4m3fn
    S = np.ascontiguousarray(np.asarray(S, dtype=np.float32))
    C = np.ascontiguousarray(np.asarray(C, dtype=np.float32))
    Wl = np.asarray(Wl, dtype=np.float32)
    Ws = np.asarray(Ws, dtype=np.float32)
    Wc = np.asarray(Wc, dtype=np.float32)
    Whs = np.asarray(Whs, dtype=np.float32)
    Whc = np.asarray(Whc, dtype=np.float32)
    fc_w = np.asarray(fc_w, dtype=np.float32)
    fc_b = np.asarray(fc_b, dtype=np.float32)

    nc = _get_nc()

    in_common = {
        "Wl": np.ascontiguousarray(Wl.astype(bf)),
        "WsT": np.ascontiguousarray(Ws.T.astype(bf)),
        "WcT": np.ascontiguousarray(Wc.T.astype(bf)),
        "whsT": np.ascontiguousarray(Whs.T.astype(bf)),
        "whcT": np.ascontiguousarray(Whc.T.astype(bf)),
        "eye2": np.eye(K, dtype=f8),
        "ident": np.eye(P, dtype=f8),
        "fcwT": np.ascontiguousarray(fc_w.T.astype(bf)),
        "fcb": np.ascontiguousarray(fc_b[None, :]),
        "ones": np.ones((P, 1), dtype=np.float32),
        "onesb": np.ones((1, P), dtype=bf),
    }
    in_maps = []
    for i in range(N_CORES):
        sl = slice(i * BPC, (i + 1) * BPC)
        in_maps.append({
            "S": np.ascontiguousarray(S[sl].astype(bf)),
            "ST": np.ascontiguousarray(S[sl].transpose(0, 2, 1).astype(bf)),
            "C": np.ascontiguousarray(C[sl].astype(bf)),
            "CT": np.ascontiguousarray(C[sl].transpose(0, 2, 1).astype(bf)),
            **in_common,
        })

    _BUILT["last_in_maps"] = in_maps
    res = run_bass_kernel_spmd(nc, in_maps, list(range(N_CORES)))
    return np.concatenate(
        [res.results[i]["out"].reshape(BPC, OUT) for i in range(N_CORES)], axis=0)


def __getattr__(name):
    if name == "_LAST_IN_MAPS":
        return _BUILT["last_in_maps"]
    raise AttributeError(name)
